# revision 1
# baseline (speedup 1.0000x reference)
"""Trainium2 Bass kernel for nn_BSplineActivationLayer.

Math:  y[b,o] = softplus( (1/OUT) * sum_i G[o,i] * f(x[b,i]; b1..b5[o,i]) )
where G = softplus(raw_gamma), b_s = piecewise-cubic spline of
w_norm = (clip(w,5.5,35.5)-20)/9, and
  f(x; b) = b1*log1p(b2*log1p((exp(b3*x)-1)**b4)) + b5*x.

Device algorithm (per core, OUT sharded 8 ways):
  * f is analytic in u = log(x) for each (o,i); interpolate it at NN fixed
    Chebyshev nodes in u.  y then becomes a sum of NN+1 matmuls over i:
       y[b,o] = softplus( (1/OUT) * [ sum_m  L_m(v[b,i]) @ N_m[o,i]
                                      + x @ (G*b5)[o,i] ] )
    with N_m = G*b1*log1p(b2*log1p((exp(b3*x_m)-1)**b4)) node values and
    L_m the Lagrange basis polys of the nodes evaluated at v = norm(log x).
  * spline eval uses expanded per-piece monomial cubics; the per-element
    piece gather is 12 masked multiply-accumulate steps per coefficient
    plane (no per-element gather hardware exists; clip() bounds prove
    pieces 0 and 14 unreachable).  Lagrange products stay f32; only the
    final matmul operands round to bf16 (single rounding).
All value-dependent math runs on device; the host only shards / transposes /
reshapes inputs and concatenates outputs.
"""

import numpy as np

B, IN, OUT = 256, 512, 512
NCORES = 8
OSH = OUT // NCORES            # 64 out-rows per core
NN = 9                         # interpolation nodes
NPIECE = 15
MU, SIG, CLO, CHI = 20.0, 9.0, 5.5, 35.5
U_LO, U_HI = float(np.log(0.01)), float(np.log(1.011))

_CACHE = {}


def _nodes():
    k = np.arange(NN)
    vn = np.cos((2 * k + 1) * np.pi / (2 * NN))          # in (-1, 1)
    xn = np.exp(0.5 * (U_HI + U_LO) + 0.5 * (U_HI - U_LO) * vn)
    cm = np.array([1.0 / np.prod(vn[m] - np.delete(vn, m)) for m in range(NN)])
    return vn, xn, cm


def _emit(ctx, tc, yT, xT, wT, rgT, ctab, brkv):
    """Emit the per-core program. All args are bass.APs of DRAM tensors.

    xT [IN, B] f32, wT/rgT [IN, OSH] f32, ctab [20, NPIECE] f32 with
    row layout k*5+s for k in (a3,a2,a1,a0), s spline; piece j innermost; brkv [1,16] f32.
    Output yT [OSH, B] f32.
    """
    import concourse.bass as bass
    from concourse import mybir

    nc = tc.nc
    f32 = mybir.dt.float32
    bf16 = mybir.dt.bfloat16
    Alu = mybir.AluOpType
    Act = mybir.ActivationFunctionType
    vn, xn, cm = _nodes()

    P = 128
    IC = IN // P                      # 4 i-chunks
    FO = IC * OSH                     # 256: free dim of (o,i)-side tiles
    FB = IC * B                       # 1024: free dim of lhs-side tiles

    pool = ctx.enter_context(tc.tile_pool(name="main", bufs=1))
    pps = ctx.enter_context(tc.tile_pool(name="ps", bufs=1, space="PSUM"))

    def bcast_mid(ap2d, n):
        """[P, F] AP -> [P, n, F] AP with 0-stride middle dim."""
        a = ap2d
        return bass.AP(tensor=a.tensor, offset=a.offset,
                       ap=[a.ap[0], [0, n], a.ap[1]])

    V = nc.vector
    S_ = nc.scalar

    CP1 = pool.tile([P, 1], f32)
    V.memset(CP1, 1.0)
    CN1 = pool.tile([P, 1], f32)
    V.memset(CN1, -1.0)

    # ---- tables ------------------------------------------------------
    BC = pool.tile([P, 20, NPIECE], f32)      # raw coef bcast (a3,a2,a1,a0 blocks)
    nc.sync.dma_start(out=BC, in_=bass.AP(
        tensor=ctab.tensor, offset=ctab.offset,
        ap=[[0, P]] + list(ctab.ap)))
    BRK = pool.tile([P, 16], f32)
    nc.sync.dma_start(out=BRK, in_=bass.AP(
        tensor=brkv.tensor, offset=brkv.offset,
        ap=[[0, P], brkv.ap[1]]))
    BETA = bcast_mid(BRK[:, 0:NPIECE], 5)     # brk_j bcast over 5 splines

    a3, a2, a1, a0 = (BC[:, 5 * k:5 * (k + 1), :] for k in range(4))
    EC = pool.tile([P, 20, NPIECE], f32)      # expanded monomial coefs
    e3, e2, e1, e0 = (EC[:, 5 * k:5 * (k + 1), :] for k in range(4))
    t1 = pool.tile([P, 5, NPIECE], f32)
    t2 = pool.tile([P, 5, NPIECE], f32)
    t3 = pool.tile([P, 5, NPIECE], f32)
    V.tensor_copy(e3, a3)
    V.tensor_mul(t1, a3, BETA)                               # a3*B
    V.scalar_tensor_tensor(e2, t1, -3.0, a2, Alu.mult, Alu.add)
    V.tensor_mul(t2, t1, BETA)                               # a3*B^2
    V.tensor_mul(t3, a2, BETA)                               # a2*B
    V.scalar_tensor_tensor(e1, t3, -2.0, a1, Alu.mult, Alu.add)
    V.scalar_tensor_tensor(e1, t2, 3.0, e1, Alu.mult, Alu.add)
    V.tensor_mul(t2, t2, BETA)                               # a3*B^3
    V.tensor_mul(t3, t3, BETA)                               # a2*B^2
    V.tensor_mul(t1, a1, BETA)                               # a1*B
    V.scalar_tensor_tensor(e0, t1, -1.0, a0, Alu.mult, Alu.add)
    V.scalar_tensor_tensor(e0, t3, 1.0, e0, Alu.mult, Alu.add)
    V.scalar_tensor_tensor(e0, t2, -1.0, e0, Alu.mult, Alu.add)
    DL = pool.tile([P, 20, NPIECE], f32)      # telescoping deltas
    V.tensor_copy(DL[:, :, 0:1], EC[:, :, 0:1])
    V.tensor_sub(DL[:, :, 1:NPIECE], EC[:, :, 1:NPIECE], EC[:, :, 0:NPIECE - 1])

    # ---- w_norm and step masks --------------------------------------
    W = pool.tile([P, FO], f32)
    nc.sync.dma_start(out=W.rearrange("p (c o) -> p c o", c=IC), in_=bass.AP(
        tensor=wT.tensor, offset=wT.offset,
        ap=[[OSH, P], [P * OSH, IC], [1, OSH]]))
    WCL = pool.tile([P, FO], f32)
    V.tensor_scalar(WCL, W, CLO, CHI, Alu.max, Alu.min)
    V.tensor_scalar(WCL, WCL, MU, 1.0 / SIG, Alu.subtract, Alu.mult)

    # clip(w,5.5,35.5) bounds wcl to [-1.612, 1.723] strictly inside
    # (brk_1, brk_14), so only steps j=2..13 can vary; piece idx is in [1,13].
    JLO, JHI = 2, 13
    NSTEP = JHI - JLO + 1
    ST = pool.tile([P, NSTEP, FO], f32)
    for j in range(JLO, JHI + 1):             # S_j = (wcl > brk_j)
        V.tensor_scalar(ST[:, j - JLO, :], WCL, BRK[:, j:j + 1], 1.0,
                        Alu.is_gt, Alu.mult)

    # ---- lhs basis inputs (independent of the w-side; emit early so
    # ACT's Ln/Copy ops overlap the DVE gather instead of gating the
    # tail) -----------------------------------------------------------
    X = pool.tile([P, FB], f32)
    nc.sync.dma_start(out=X.rearrange("p (c b) -> p c b", c=IC), in_=bass.AP(
        tensor=xT.tensor, offset=xT.offset,
        ap=[[B, P], [P * B, IC], [1, B]]))
    VT = pool.tile([P, FB], f32)
    S_.activation(VT, X, Act.Ln)
    V.tensor_scalar(VT, VT, 2.0 / (U_HI - U_LO), (U_HI + U_LO) / (U_HI - U_LO),
                    Alu.mult, Alu.subtract)
    DD = pool.tile([P, NN, FB], f32)
    for m in range(NN):
        S_.activation(DD[:, m, :], VT, Act.Copy, bias=float(-vn[m]))

    # ---- gamma (independent; emit early so ACT overlaps the gather) --
    RG = pool.tile([P, FO], f32)
    nc.sync.dma_start(out=RG.rearrange("p (c o) -> p c o", c=IC), in_=bass.AP(
        tensor=rgT.tensor, offset=rgT.offset,
        ap=[[OSH, P], [P * OSH, IC], [1, OSH]]))
    G = pool.tile([P, FO], f32)
    S_.activation(G, RG, Act.Exp)
    S_.activation(G, G, Act.Ln, bias=CP1)     # softplus(rg)

    # ---- piece gather (20 planes) + Horner, spline-ordered ----------
    # (walrus rejects TensorScalarPtr/TensorTensor on the Pool engine, so
    # the gather stays on DVE.)  Splines ordered b3,b4,b2,b1,b5 so the
    # ACT node chains can start while the gather is still running.
    A = pool.tile([P, 20, FO], f32)
    BP = pool.tile([P, 5, FO], f32)
    E = pool.tile([P, NN, FO], f32)
    GB1 = pool.tile([P, FO], f32)
    GB5 = pool.tile([P, FO], f32)
    for s in (2, 3, 1, 0, 4):
        for p in (s, 5 + s, 10 + s, 15 + s):
            V.tensor_scalar(A[:, p, :], ST[:, 0, :], DL[:, p, JLO:JLO + 1],
                            EC[:, p, 1:2], Alu.mult, Alu.add)
            for j in range(JLO + 1, JHI + 1):
                V.scalar_tensor_tensor(A[:, p, :], ST[:, j - JLO, :],
                                       DL[:, p, j:j + 1], A[:, p, :],
                                       Alu.mult, Alu.add)
        h = BP[:, s, :]
        V.tensor_mul(h, A[:, s, :], WCL)
        V.tensor_add(h, h, A[:, 5 + s, :])
        V.tensor_mul(h, h, WCL)
        V.tensor_add(h, h, A[:, 10 + s, :])
        V.tensor_mul(h, h, WCL)
        V.tensor_add(h, h, A[:, 15 + s, :])
        if s == 2:                       # b3 ready: launch the Exp chains
            for m in range(NN):
                S_.activation(E[:, m, :], BP[:, 2, :], Act.Exp,
                              scale=float(xn[m]))
        elif s == 0:
            V.tensor_mul(GB1, G, BP[:, 0, :])
        elif s == 4:
            V.tensor_mul(GB5, G, BP[:, 4, :])

    # ---- node-value chains  N_m = G*b1*log1p(b2*log1p((e^{b3 x_m}-1)^b4))
    EF = E.rearrange("p n f -> p (n f)")
    S_.activation(EF, EF, Act.Ln, bias=CN1)
    V.tensor_mul(E, E, bcast_mid(BP[:, 3, :], NN))
    S_.activation(EF, EF, Act.Exp)
    S_.activation(EF, EF, Act.Ln, bias=CP1)
    V.tensor_mul(E, E, bcast_mid(BP[:, 1, :], NN))
    S_.activation(EF, EF, Act.Ln, bias=CP1)
    EN = pool.tile([P, NN, FO], bf16)
    V.tensor_mul(EN, E, bcast_mid(GB1, NN))

    # ---- lhs basis: products (inputs built early, above) ------------
    LL = pool.tile([P, NN, FB], f32)
    LB = pool.tile([P, NN, FB], bf16)
    V.tensor_mul(LL[:, 2, :], DD[:, 0, :], DD[:, 1, :])
    for m in range(3, NN):
        V.tensor_mul(LL[:, m, :], LL[:, m - 1, :], DD[:, m - 1, :])
    SFX = pool.tile([P, FB], f32)
    V.tensor_scalar(LB[:, NN - 1, :], LL[:, NN - 1, :], float(cm[NN - 1]), 1.0,
                    Alu.mult, Alu.mult)
    V.scalar_tensor_tensor(LB[:, NN - 2, :], LL[:, NN - 2, :], float(cm[NN - 2]),
                           DD[:, NN - 1, :], Alu.mult, Alu.mult)
    V.tensor_mul(SFX, DD[:, NN - 1, :], DD[:, NN - 2, :])
    for m in range(NN - 3, 0, -1):
        prefix = LL[:, m, :] if m >= 2 else DD[:, 0, :]
        V.scalar_tensor_tensor(LB[:, m, :], prefix, float(cm[m]), SFX,
                               Alu.mult, Alu.mult)
        if m > 1:
            V.tensor_mul(SFX, SFX, DD[:, m, :])
    V.tensor_mul(SFX, SFX, DD[:, 1, :])
    V.tensor_scalar(LB[:, 0, :], SFX, float(cm[0]), 1.0, Alu.mult, Alu.mult)

    # ---- matmuls ----------------------------------------------------
    ps = pps.tile([OSH, B], f32)
    nmm = IC * (NN + 1)
    k = 0
    for ic in range(IC):
        nc.tensor.matmul(ps, GB5[:, ic * OSH:(ic + 1) * OSH],
                         X[:, ic * B:(ic + 1) * B],
                         start=(k == 0), stop=(k == nmm - 1))
        k += 1
    for m in range(NN):
        for ic in range(IC):
            nc.tensor.matmul(ps, EN[:, m, ic * OSH:(ic + 1) * OSH],
                             LB[:, m, ic * B:(ic + 1) * B],
                             start=(k == 0), stop=(k == nmm - 1))
            k += 1

    # ---- softplus + store -------------------------------------------
    Y = pool.tile([OSH, B], f32)
    S_.activation(Y, ps, Act.Exp, scale=1.0 / OUT)
    S_.activation(Y, Y, Act.Ln, bias=CP1[0:OSH, :])
    nc.sync.dma_start(out=yT, in_=Y)


def _build():
    if "nc" in _CACHE:
        return _CACHE["nc"]
    from contextlib import ExitStack
    import concourse.bacc as bacc
    import concourse.tile as tile
    from concourse import mybir

    f32 = mybir.dt.float32
    nc = bacc.Bacc("TRN2", target_bir_lowering=False, debug=False,
                   num_devices=NCORES)
    xT = nc.dram_tensor("xT", [IN, B], f32, kind="ExternalInput").ap()
    wT = nc.dram_tensor("wT", [IN, OSH], f32, kind="ExternalInput").ap()
    rgT = nc.dram_tensor("rgT", [IN, OSH], f32, kind="ExternalInput").ap()
    ctab = nc.dram_tensor("ctab", [20, NPIECE], f32, kind="ExternalInput").ap()
    brkv = nc.dram_tensor("brkv", [1, 16], f32, kind="ExternalInput").ap()
    yT = nc.dram_tensor("yT", [OSH, B], f32, kind="ExternalOutput").ap()

    with tile.TileContext(nc) as tc, ExitStack() as ctx:
        _emit(ctx, tc, yT, xT, wT, rgT, ctab, brkv)
    nc.compile()
    _CACHE["nc"] = nc
    return nc


def _prep_inputs(x, raw_gamma, w, breaks, coefs):
    xT = np.ascontiguousarray(x.T, dtype=np.float32)
    ctab = np.ascontiguousarray(
        coefs.transpose(2, 0, 1).reshape(20, NPIECE), dtype=np.float32)
    brkv = np.ascontiguousarray(breaks[0:1, :], dtype=np.float32)
    maps = []
    for c in range(NCORES):
        o0, o1 = c * OSH, (c + 1) * OSH
        maps.append({
            "xT": xT,
            "wT": np.ascontiguousarray(w[o0:o1].T, dtype=np.float32),
            "rgT": np.ascontiguousarray(raw_gamma[o0:o1].T, dtype=np.float32),
            "ctab": ctab,
            "brkv": brkv,
        })
    return maps


def kernel(x, raw_gamma, w, breaks, coefs):
    from concourse.bass_utils import run_bass_kernel_spmd
    nc = _build()
    maps = _prep_inputs(x, raw_gamma, w, breaks, coefs)
    res = run_bass_kernel_spmd(nc, maps, list(range(NCORES)))
    y = np.concatenate([res.results[c]["yT"].T for c in range(NCORES)], axis=1)
    return np.ascontiguousarray(y, dtype=np.float32)



# revision 3
# speedup vs baseline: 2.5355x; 2.5355x over previous
"""Trainium2 Bass kernel for nn_BSplineActivationLayer.

Math:  y[b,o] = softplus( (1/OUT) * sum_i G[o,i] * f(x[b,i]; b1..b5[o,i]) )
where G = softplus(raw_gamma), b_s = piecewise-cubic spline of
w_norm = (clip(w,5.5,35.5)-20)/9, and
  f(x; b) = b1*log1p(b2*log1p((exp(b3*x)-1)**b4)) + b5*x.

Device algorithm (per core, OUT sharded 8 ways), tuned to the 2e-2 rel-err
budget (measured end-to-end error ~2e-3):
  * spline b_s is approximated piecewise-CONSTANT per piece (value of the
    cubic at the piece midpoint t=0.125); the per-element piece gather is a
    12-step masked multiply-accumulate per spline with the table values
    baked into the instruction stream as immediates (compile happens after
    inputs are seen; cache keyed on the table bytes).  clip() bounds prove
    pieces 0 and 14 unreachable, and the breaks are uniform so the masks
    compare raw w against MU+SIG*brk_j directly -- no normalization ops.
  * f is analytic in u = log(x); interpolate at NN=4 Chebyshev nodes in u:
      y[b,o] = softplus( (1/OUT) * [ sum_m  L_m(v[b,i]) @ N_m[o,i]
                                     + x @ (G*b5)[o,i] ] )
    with N_m = G*b1*cm_m*log1p(b2*log1p((exp(b3*x_m)-1)**b4)) node values
    and L_m the (unscaled) Lagrange basis products of v = norm(log x).
  * work is balanced across DVE (3 gather planes, chain muls, Lagrange
    finals, EN), Pool/GPSIMD (step masks, 2 gather planes, DD, products,
    gammas -- walrus accepts TensorTensor and immediate TensorScalar on
    Pool), and ACT (one manually placed set-6 table load serves every
    exp/ln/copy).  Matmul operands round to bf16 except the x-term, which
    stays f32 (PE has slack).  Junk matmuls keep PE ramped before the tail.
All value-dependent math on the big tensors runs on device; the host only
shards / transposes inputs, prepares the tiny (5x15) spline table constants,
and concatenates outputs.
"""

import numpy as np

B, IN, OUT = 256, 512, 512
NCORES = 8
OSH = OUT // NCORES            # 64 out-rows per core
NN = 4                         # interpolation nodes
NPIECE = 15
MU, SIG = 20.0, 9.0
U_LO, U_HI = float(np.log(0.01)), float(np.log(1.011))
TM = 0.125                     # piece-midpoint for the constant approx
JLO, JHI = 2, 13               # reachable step boundaries
NWARM = 0                      # junk matmuls to keep PE ramped (tuned)

_CACHE = {}


def _nodes():
    k = np.arange(NN)
    vn = np.cos((2 * k + 1) * np.pi / (2 * NN))          # in (-1, 1)
    xn = np.exp(0.5 * (U_HI + U_LO) + 0.5 * (U_HI - U_LO) * vn)
    cm = np.array([1.0 / np.prod(vn[m] - np.delete(vn, m)) for m in range(NN)])
    return vn, xn, cm


def _tables(breaks, coefs):
    """Host prep of the small spline tables -> immediates.

    Returns thr[j] (mask thresholds in raw-w domain, j=JLO..JHI),
    base[s], delta[s][j] for the piecewise-constant masked accumulate."""
    brk = breaks[0].astype(np.float64)
    cf = coefs.astype(np.float64)
    a3, a2, a1, a0 = cf[..., 0], cf[..., 1], cf[..., 2], cf[..., 3]
    vmid = ((a3 * TM + a2) * TM + a1) * TM + a0          # [NS, K]
    thr = MU + SIG * brk                                  # [16]
    base = vmid[:, 1]
    delta = vmid[:, 1:] - vmid[:, :-1]                    # delta[s, j-1] = v_j - v_{j-1}
    return thr, base, vmid, delta


def _emit(ctx, tc, yT, xT, wT, rgT, thr, base, delta):
    import concourse.bass as bass
    from concourse import mybir

    nc = tc.nc
    f32 = mybir.dt.float32
    bf16 = mybir.dt.bfloat16
    Alu = mybir.AluOpType
    Act = mybir.ActivationFunctionType
    vn, xn, cm = _nodes()

    P = 128
    IC = IN // P                      # 4 i-chunks
    FO = IC * OSH                     # 256
    FB = IC * B                       # 1024

    pool = ctx.enter_context(tc.tile_pool(name="main", bufs=1))
    pps = ctx.enter_context(tc.tile_pool(name="ps", bufs=1, space="PSUM"))

    def bcast_mid(ap2d, n):
        a = ap2d
        return bass.AP(tensor=a.tensor, offset=a.offset,
                       ap=[a.ap[0], [0, n], a.ap[1]])

    V = nc.vector
    Pl = nc.gpsimd
    S_ = nc.scalar

    ascale = 2.0 / (U_HI - U_LO)
    boff = (U_HI + U_LO) / (U_HI - U_LO)

    # ---- constants ---------------------------------------------------
    CP1 = pool.tile([P, 1], f32)
    V.memset(CP1, 1.0)
    CN1 = pool.tile([P, 1], f32)
    V.memset(CN1, -1.0)

    # ---- manual act-table load: set 6 covers exp/ln/copy -------------
    atl = mybir.InstLoadActFuncSet(
        name=nc.get_next_instruction_name(), act_func_set_id=6, ins=[], outs=[])
    S_.add_instruction(atl)

    # ---- DMAs --------------------------------------------------------
    W = pool.tile([P, FO], f32)
    nc.sync.dma_start(out=W.rearrange("p (c o) -> p c o", c=IC), in_=bass.AP(
        tensor=wT.tensor, offset=wT.offset,
        ap=[[OSH, P], [P * OSH, IC], [1, OSH]]))
    X = pool.tile([P, IC, B], f32)
    nc.sync.dma_start(out=X, in_=bass.AP(
        tensor=xT.tensor, offset=xT.offset,
        ap=[[B, P], [P * B, IC], [1, B]]))
    RG = pool.tile([P, FO], f32)
    nc.sync.dma_start(out=RG.rearrange("p (c o) -> p c o", c=IC), in_=bass.AP(
        tensor=rgT.tensor, offset=rgT.offset,
        ap=[[OSH, P], [P * OSH, IC], [1, OSH]]))

    # ---- step masks (Pool) ------------------------------------------
    NSTEP = JHI - JLO + 1             # 12
    ST = pool.tile([P, NSTEP, FO], f32)
    for j in range(JLO, JHI + 1):
        Pl.tensor_scalar(ST[:, j - JLO, :], W, float(thr[j]), 1.0,
                         Alu.is_gt, Alu.mult)

    # ---- gamma + log(x) (ACT) ---------------------------------------
    G = pool.tile([P, FO], f32)
    S_.activation(G, RG, Act.Exp)
    S_.activation(G, G, Act.Ln, bias=CP1)         # softplus(rg)
    U = pool.tile([P, IC, B], f32)
    XF = X.rearrange("p c b -> p (c b)")
    UF = U.rearrange("p c b -> p (c b)")
    S_.activation(UF, XF, Act.Ln)

    # ---- gather planes: DVE does b3,b4,b2 (s=2,3,1); Pool b1,b5 ------
    A = [pool.tile([P, FO], f32, name=f"A{s}") for s in range(5)]
    for s in (2, 3, 1):
        V.tensor_scalar(A[s], ST[:, 0, :], float(delta[s, JLO - 1]),
                        float(base[s]), Alu.mult, Alu.add)
        for j in range(JLO + 1, JHI + 1):
            V.scalar_tensor_tensor(A[s], ST[:, j - JLO, :],
                                   float(delta[s, j - 1]), A[s],
                                   Alu.mult, Alu.add)
        if s == 2:
            # b3 ready: launch node exponentials on ACT
            E = pool.tile([P, NN, FO], f32)
            for m in range(NN):
                S_.activation(E[:, m, :], A[2], Act.Exp, scale=float(xn[m]))
            EF = E.rearrange("p n f -> p (n f)")
            S_.activation(EF, EF, Act.Ln, bias=CN1)       # lam = ln(e^{b3 xm}-1)
        elif s == 3:
            # T = lam * b4  (DVE, bcast over nodes)
            V.tensor_tensor(E, E, bcast_mid(A[3], NN), Alu.mult)
            S_.activation(EF, EF, Act.Exp)                # (e^{b3 xm}-1)^{b4}
            S_.activation(EF, EF, Act.Ln, bias=CP1)       # L1 = log1p(...)
        elif s == 1:
            V.tensor_tensor(E, E, bcast_mid(A[1], NN), Alu.mult)  # b2*L1
            EB = pool.tile([P, NN, FO], bf16)
            S_.activation(EB.rearrange("p n f -> p (n f)"), EF,
                          Act.Ln, bias=CP1)               # L2 = log1p(b2 L1)

    # Pool planes with mask*delta fused in the TS op
    TMP = pool.tile([P, 2, FO], f32)
    for k, s in enumerate((0, 4)):
        Pl.tensor_scalar(A[s], ST[:, 0, :], float(delta[s, JLO - 1]),
                         float(base[s]), Alu.mult, Alu.add)
        for j in range(JLO + 1, JHI + 1):
            Pl.tensor_scalar(TMP[:, k, :], ST[:, j - JLO, :],
                             float(delta[s, j - 1]), 1.0, Alu.mult, Alu.mult)
            Pl.tensor_tensor(A[s], A[s], TMP[:, k, :], Alu.add)
        if s == 0:
            GB1 = pool.tile([P, FO], f32)
            Pl.tensor_tensor(GB1, G, A[0], Alu.mult)
        else:
            GB5 = pool.tile([P, FO], f32)
            Pl.tensor_tensor(GB5, G, A[4], Alu.mult)

    # ---- DD_m = v - vn_m on Pool (bf16) ------------------------------
    DD = pool.tile([P, NN, FB], bf16)
    for m in range(NN):
        Pl.tensor_scalar(DD[:, m, :], UF, ascale, boff + float(vn[m]),
                         Alu.mult, Alu.subtract)

    # ---- Lagrange products: pairs on Pool, finals on DVE (bf16 2x) ---
    P01 = pool.tile([P, FB], bf16)
    P23 = pool.tile([P, FB], bf16)
    Pl.tensor_tensor(P01, DD[:, 0, :], DD[:, 1, :], Alu.mult)
    Pl.tensor_tensor(P23, DD[:, 2, :], DD[:, 3, :], Alu.mult)
    LB = pool.tile([P, NN, FB], bf16)
    V.tensor_tensor(LB[:, 0, :], DD[:, 1, :], P23, Alu.mult)
    V.tensor_tensor(LB[:, 1, :], DD[:, 0, :], P23, Alu.mult)
    V.tensor_tensor(LB[:, 2, :], P01, DD[:, 3, :], Alu.mult)
    V.tensor_tensor(LB[:, 3, :], P01, DD[:, 2, :], Alu.mult)

    # ---- GCM_m = G*b1*cm_m (Pool), EN = L2 * GCM (DVE bf16 2x) -------
    GCM = pool.tile([P, NN, FO], bf16)
    for m in range(NN):
        Pl.tensor_scalar(GCM[:, m, :], GB1, float(cm[m]), 1.0,
                         Alu.mult, Alu.mult)
    EN = pool.tile([P, NN, FO], bf16)
    V.tensor_tensor(EN, EB, GCM, Alu.mult)

    # ---- matmuls -----------------------------------------------------
    ps = pps.tile([OSH, B], f32)
    if NWARM:
        psj = pps.tile([OSH, B], f32)
        ZJ = pool.tile([P, OSH], bf16)
        ZM = pool.tile([P, B], bf16)
        V.memset(ZJ, 0.0)
        V.memset(ZM, 0.0)
        for k in range(NWARM):
            nc.tensor.matmul(psj, ZJ, ZM, start=(k == 0), stop=(k == NWARM - 1))
    nmm = IC * (NN + 1)
    k = 0
    GB5v = GB5.rearrange("p (c o) -> p c o", c=IC)
    for ic in range(IC):
        nc.tensor.matmul(ps, GB5v[:, ic, :], X[:, ic, :],
                         start=(k == 0), stop=(k == nmm - 1))
        k += 1
    ENv = EN.rearrange("p n (c o) -> p n c o", c=IC)
    LBv = LB.rearrange("p n (c b) -> p n c b", c=IC)
    for m in range(NN):
        for ic in range(IC):
            nc.tensor.matmul(ps, ENv[:, m, ic, :], LBv[:, m, ic, :],
                             start=(k == 0), stop=(k == nmm - 1))
            k += 1

    # ---- softplus + store -------------------------------------------
    Y = pool.tile([OSH, B], f32)
    S_.activation(Y, ps, Act.Exp, scale=1.0 / OUT)
    S_.activation(Y, Y, Act.Ln, bias=CP1[0:OSH, :])
    nc.sync.dma_start(out=yT, in_=Y)


def _build(breaks, coefs):
    key = (breaks.tobytes(), coefs.tobytes())
    if key in _CACHE:
        return _CACHE[key]
    from contextlib import ExitStack
    import concourse.bacc as bacc
    import concourse.tile as tile
    from concourse import mybir

    thr, base, vmid, delta = _tables(breaks, coefs)

    f32 = mybir.dt.float32
    nc = bacc.Bacc("TRN2", target_bir_lowering=False, debug=False,
                   num_devices=NCORES)
    xT = nc.dram_tensor("xT", [IN, B], f32, kind="ExternalInput").ap()
    wT = nc.dram_tensor("wT", [IN, OSH], f32, kind="ExternalInput").ap()
    rgT = nc.dram_tensor("rgT", [IN, OSH], f32, kind="ExternalInput").ap()
    yT = nc.dram_tensor("yT", [OSH, B], f32, kind="ExternalOutput").ap()

    with tile.TileContext(nc) as tc, ExitStack() as ctx:
        _emit(ctx, tc, yT, xT, wT, rgT, thr, base, delta)
    nc.compile()
    _CACHE[key] = nc
    return nc


def _prep_inputs(x, raw_gamma, w, breaks, coefs):
    xT = np.ascontiguousarray(x.T, dtype=np.float32)
    maps = []
    for c in range(NCORES):
        o0, o1 = c * OSH, (c + 1) * OSH
        maps.append({
            "xT": xT,
            "wT": np.ascontiguousarray(w[o0:o1].T, dtype=np.float32),
            "rgT": np.ascontiguousarray(raw_gamma[o0:o1].T, dtype=np.float32),
        })
    return maps


def kernel(x, raw_gamma, w, breaks, coefs):
    from concourse.bass_utils import run_bass_kernel_spmd
    nc = _build(np.asarray(breaks), np.asarray(coefs))
    maps = _prep_inputs(x, raw_gamma, w, breaks, coefs)
    res = run_bass_kernel_spmd(nc, maps, list(range(NCORES)))
    y = np.concatenate([res.results[c]["yT"].T for c in range(NCORES)], axis=1)
    return np.ascontiguousarray(y, dtype=np.float32)


# revision 4
# speedup vs baseline: 3.7701x; 1.4869x over previous
"""Trainium2 Bass kernel for nn_BSplineActivationLayer.

Math:  y[b,o] = softplus( (1/OUT) * sum_i G[o,i] * f(x[b,i]; b1..b5[o,i]) )
where G = softplus(raw_gamma), b_s = piecewise-cubic spline of
w_norm = (clip(w,5.5,35.5)-20)/9, and
  f(x; b) = b1*log1p(b2*log1p((exp(b3*x)-1)**b4)) + b5*x.

Device algorithm (per core, OUT sharded 8 ways), tuned to the 2e-2 rel-err
budget (measured end-to-end error ~2e-3):
  * spline b_s is approximated piecewise-CONSTANT per piece (value of the
    cubic at the piece midpoint t=0.125); the per-element piece gather is a
    12-step masked multiply-accumulate per spline with the table values
    baked into the instruction stream as immediates (compile happens after
    inputs are seen; cache keyed on the table bytes).  clip() bounds prove
    pieces 0 and 14 unreachable, and the breaks are uniform so the masks
    compare raw w against MU+SIG*brk_j directly -- no normalization ops.
  * f is analytic in u = log(x); interpolate at NN=4 Chebyshev nodes in u:
      y[b,o] = softplus( (1/OUT) * [ sum_m  L_m(v[b,i]) @ N_m[o,i]
                                     + x @ (G*b5)[o,i] ] )
    with N_m = G*b1*cm_m*log1p(b2*log1p((exp(b3*x_m)-1)**b4)) node values
    and L_m the (unscaled) Lagrange basis products of v = norm(log x).
  * work is balanced across DVE (3 gather planes, chain muls, Lagrange
    finals, EN), Pool/GPSIMD (step masks, 2 gather planes, DD, products,
    gammas -- walrus accepts TensorTensor and immediate TensorScalar on
    Pool), and ACT (one manually placed set-6 table load serves every
    exp/ln/copy).  Matmul operands round to bf16 except the x-term, which
    stays f32 (PE has slack).  Junk matmuls keep PE ramped before the tail.
All value-dependent math on the big tensors runs on device; the host only
shards / transposes inputs, prepares the tiny (5x15) spline table constants,
and concatenates outputs.
"""

import numpy as np

B, IN, OUT = 256, 512, 512
NCORES = 8
OSH = OUT // NCORES            # 64 out-rows per core
NN = 4                         # interpolation nodes
NPIECE = 15
MU, SIG = 20.0, 9.0
U_LO, U_HI = float(np.log(0.01)), float(np.log(1.011))
TM = 0.125                     # piece-midpoint for the constant approx
JLO, JHI = 2, 13               # reachable step boundaries
NWARM = 0                      # junk matmuls to keep PE ramped (tuned)

_CACHE = {}


def _nodes():
    k = np.arange(NN)
    vn = np.cos((2 * k + 1) * np.pi / (2 * NN))          # in (-1, 1)
    xn = np.exp(0.5 * (U_HI + U_LO) + 0.5 * (U_HI - U_LO) * vn)
    cm = np.array([1.0 / np.prod(vn[m] - np.delete(vn, m)) for m in range(NN)])
    return vn, xn, cm


def _tables(breaks, coefs):
    """Host prep of the small spline tables -> immediates.

    Returns thr[j] (mask thresholds in raw-w domain, j=JLO..JHI),
    base[s], delta[s][j] for the piecewise-constant masked accumulate."""
    brk = breaks[0].astype(np.float64)
    cf = coefs.astype(np.float64)
    a3, a2, a1, a0 = cf[..., 0], cf[..., 1], cf[..., 2], cf[..., 3]
    vmid = ((a3 * TM + a2) * TM + a1) * TM + a0          # [NS, K]
    thr = MU + SIG * brk                                  # [16]
    base = vmid[:, 1]
    delta = vmid[:, 1:] - vmid[:, :-1]                    # delta[s, j-1] = v_j - v_{j-1}
    return thr, base, vmid, delta


def _emit(ctx, tc, yT, xT, wT, rgT, thr, base, delta):
    import concourse.bass as bass
    from concourse import mybir

    nc = tc.nc
    f32 = mybir.dt.float32
    bf16 = mybir.dt.bfloat16
    Alu = mybir.AluOpType
    Act = mybir.ActivationFunctionType
    vn, xn, cm = _nodes()

    P = 128
    IC = IN // P                      # 4 i-chunks
    FO = IC * OSH                     # 256
    FB = IC * B                       # 1024

    pool = ctx.enter_context(tc.tile_pool(name="main", bufs=1))
    pps = ctx.enter_context(tc.tile_pool(name="ps", bufs=1, space="PSUM"))

    def bcast_mid(ap2d, n):
        a = ap2d
        return bass.AP(tensor=a.tensor, offset=a.offset,
                       ap=[a.ap[0], [0, n], a.ap[1]])

    V = nc.vector
    Pl = nc.gpsimd
    S_ = nc.scalar

    ascale = 2.0 / (U_HI - U_LO)
    boff = (U_HI + U_LO) / (U_HI - U_LO)

    # ---- constants ---------------------------------------------------
    CP1 = pool.tile([P, 1], f32)
    V.memset(CP1, 1.0)
    CN1 = pool.tile([P, 1], f32)
    V.memset(CN1, -1.0)

    # ---- manual act-table load: set 6 covers exp/ln/copy -------------
    atl = mybir.InstLoadActFuncSet(
        name=nc.get_next_instruction_name(), act_func_set_id=6, ins=[], outs=[])
    S_.add_instruction(atl)

    # ---- DMAs --------------------------------------------------------
    W = pool.tile([P, FO], f32)
    nc.sync.dma_start(out=W.rearrange("p (c o) -> p c o", c=IC), in_=bass.AP(
        tensor=wT.tensor, offset=wT.offset,
        ap=[[OSH, P], [P * OSH, IC], [1, OSH]]))
    X = pool.tile([P, IC, B], f32)
    nc.sync.dma_start(out=X, in_=bass.AP(
        tensor=xT.tensor, offset=xT.offset,
        ap=[[B, P], [P * B, IC], [1, B]]))
    RG = pool.tile([P, FO], f32)
    nc.sync.dma_start(out=RG.rearrange("p (c o) -> p c o", c=IC), in_=bass.AP(
        tensor=rgT.tensor, offset=rgT.offset,
        ap=[[OSH, P], [P * OSH, IC], [1, OSH]]))

    # bf16 copy of x for the b5-term matmul via casting SWDGE DMA (Pool)
    XB = pool.tile([P, IC, B], bf16)
    Pl.dma_start(out=XB, in_=bass.AP(
        tensor=xT.tensor, offset=xT.offset,
        ap=[[B, P], [P * B, IC], [1, B]]))

    # ---- step masks (Pool) ------------------------------------------
    NSTEP = JHI - JLO + 1             # 12
    ST = pool.tile([P, NSTEP, FO], f32)
    for j in range(JLO, JHI + 1):
        Pl.tensor_scalar(ST[:, j - JLO, :], W, float(thr[j]), 1.0,
                         Alu.is_gt, Alu.mult)

    # ---- gamma + log(x) (ACT) ---------------------------------------
    G = pool.tile([P, FO], f32)
    S_.activation(G, RG, Act.Exp)
    S_.activation(G, G, Act.Ln, bias=CP1)         # softplus(rg)
    U = pool.tile([P, IC, B], f32)
    XF = X.rearrange("p c b -> p (c b)")
    UF = U.rearrange("p c b -> p (c b)")
    S_.activation(UF, XF, Act.Ln)

    # ---- DD_m = v - vn_m on ACT (bf16) -------------------------------
    DD = pool.tile([P, NN, FB], bf16)
    for m in range(NN):
        S_.activation(DD[:, m, :], UF, Act.Copy, scale=ascale,
                      bias=-(boff + float(vn[m])))

    # ---- all 5 gather planes on DVE (TSP has no perf modes; DVE is the
    # cheapest engine for the masked accumulate).  Chain-feeding planes
    # first (b3,b4,b2), then b1 (gamma path), then b5 (x-term). ---------
    A = [pool.tile([P, FO], f32, name=f"A{s}") for s in range(5)]
    E = pool.tile([P, NN, FO], f32)
    EF = E.rearrange("p n f -> p (n f)")
    EB = pool.tile([P, NN, FO], bf16)
    for s in (2, 3, 1, 0, 4):
        V.tensor_scalar(A[s], ST[:, 0, :], float(delta[s, JLO - 1]),
                        float(base[s]), Alu.mult, Alu.add)
        for j in range(JLO + 1, JHI + 1):
            V.scalar_tensor_tensor(A[s], ST[:, j - JLO, :],
                                   float(delta[s, j - 1]), A[s],
                                   Alu.mult, Alu.add)
        if s == 2:
            # b3 ready: node exponentials + lam on ACT
            for m in range(NN):
                S_.activation(E[:, m, :], A[2], Act.Exp, scale=float(xn[m]))
            S_.activation(EF, EF, Act.Ln, bias=CN1)       # lam = ln(e^{b3 xm}-1)
        elif s == 3:
            # T = lam * b4  (Pool, bcast over nodes)
            Pl.tensor_tensor(E, E, bcast_mid(A[3], NN), Alu.mult)
            S_.activation(EF, EF, Act.Exp)                # (e^{b3 xm}-1)^{b4}
            S_.activation(EF, EF, Act.Ln, bias=CP1)       # L1 = log1p(...)
        elif s == 1:
            Pl.tensor_tensor(E, E, bcast_mid(A[1], NN), Alu.mult)  # b2*L1
            S_.activation(EB.rearrange("p n f -> p (n f)"), EF,
                          Act.Ln, bias=CP1)               # L2 = log1p(b2 L1)
        elif s == 0:
            GB1 = pool.tile([P, FO], f32)
            Pl.tensor_tensor(GB1, G, A[0], Alu.mult)
            GCM = pool.tile([P, NN, FO], bf16)
            for m in range(NN):
                Pl.tensor_scalar(GCM[:, m, :], GB1, float(cm[m]), 1.0,
                                 Alu.mult, Alu.mult)
        else:
            GB5 = pool.tile([P, FO], bf16)
            Pl.tensor_tensor(GB5, G, A[4], Alu.mult)

    # ---- Lagrange products: pairs on Pool (early), finals on DVE -----
    P01 = pool.tile([P, FB], bf16)
    P23 = pool.tile([P, FB], bf16)
    Pl.tensor_tensor(P01, DD[:, 0, :], DD[:, 1, :], Alu.mult)
    Pl.tensor_tensor(P23, DD[:, 2, :], DD[:, 3, :], Alu.mult)
    LB = pool.tile([P, NN, FB], bf16)
    V.tensor_tensor(LB[:, 0, :], DD[:, 1, :], P23, Alu.mult)
    V.tensor_tensor(LB[:, 1, :], DD[:, 0, :], P23, Alu.mult)
    V.tensor_tensor(LB[:, 2, :], P01, DD[:, 3, :], Alu.mult)
    V.tensor_tensor(LB[:, 3, :], P01, DD[:, 2, :], Alu.mult)

    # ---- matmuls; EN split per node so PE starts as soon as possible -
    ps = pps.tile([OSH, B], f32)
    if NWARM:
        psj = pps.tile([OSH, B], f32)
        ZJ = pool.tile([P, OSH], bf16)
        ZM = pool.tile([P, B], bf16)
        V.memset(ZJ, 0.0)
        V.memset(ZM, 0.0)
        for k in range(NWARM):
            nc.tensor.matmul(psj, ZJ, ZM, start=(k == 0), stop=(k == NWARM - 1))
    nmm = IC * (NN + 1)
    k = 0
    GB5v = GB5.rearrange("p (c o) -> p c o", c=IC)
    for ic in range(IC):
        nc.tensor.matmul(ps, GB5v[:, ic, :], XB[:, ic, :],
                         start=(k == 0), stop=(k == nmm - 1))
        k += 1
    EN = pool.tile([P, NN, FO], bf16)
    ENv = EN.rearrange("p n (c o) -> p n c o", c=IC)
    LBv = LB.rearrange("p n (c b) -> p n c b", c=IC)
    for m in range(NN):
        V.tensor_tensor(EN[:, m, :], EB[:, m, :], GCM[:, m, :], Alu.mult)
        for ic in range(IC):
            nc.tensor.matmul(ps, ENv[:, m, ic, :], LBv[:, m, ic, :],
                             start=(k == 0), stop=(k == nmm - 1))
            k += 1

    # ---- softplus + store -------------------------------------------
    Y = pool.tile([OSH, B], f32)
    S_.activation(Y, ps, Act.Exp, scale=1.0 / OUT)
    S_.activation(Y, Y, Act.Ln, bias=CP1[0:OSH, :])
    nc.sync.dma_start(out=yT, in_=Y)


def _build(breaks, coefs):
    key = (breaks.tobytes(), coefs.tobytes())
    if key in _CACHE:
        return _CACHE[key]
    from contextlib import ExitStack
    import concourse.bacc as bacc
    import concourse.tile as tile
    from concourse import mybir

    thr, base, vmid, delta = _tables(breaks, coefs)

    f32 = mybir.dt.float32
    nc = bacc.Bacc("TRN2", target_bir_lowering=False, debug=False,
                   num_devices=NCORES)
    xT = nc.dram_tensor("xT", [IN, B], f32, kind="ExternalInput").ap()
    wT = nc.dram_tensor("wT", [IN, OSH], f32, kind="ExternalInput").ap()
    rgT = nc.dram_tensor("rgT", [IN, OSH], f32, kind="ExternalInput").ap()
    yT = nc.dram_tensor("yT", [OSH, B], f32, kind="ExternalOutput").ap()

    with tile.TileContext(nc) as tc, ExitStack() as ctx:
        _emit(ctx, tc, yT, xT, wT, rgT, thr, base, delta)
    nc.compile()
    _CACHE[key] = nc
    return nc


def _prep_inputs(x, raw_gamma, w, breaks, coefs):
    xT = np.ascontiguousarray(x.T, dtype=np.float32)
    maps = []
    for c in range(NCORES):
        o0, o1 = c * OSH, (c + 1) * OSH
        maps.append({
            "xT": xT,
            "wT": np.ascontiguousarray(w[o0:o1].T, dtype=np.float32),
            "rgT": np.ascontiguousarray(raw_gamma[o0:o1].T, dtype=np.float32),
        })
    return maps


def kernel(x, raw_gamma, w, breaks, coefs):
    from concourse.bass_utils import run_bass_kernel_spmd
    nc = _build(np.asarray(breaks), np.asarray(coefs))
    maps = _prep_inputs(x, raw_gamma, w, breaks, coefs)
    res = run_bass_kernel_spmd(nc, maps, list(range(NCORES)))
    y = np.concatenate([res.results[c]["yT"].T for c in range(NCORES)], axis=1)
    return np.ascontiguousarray(y, dtype=np.float32)


# revision 8
# speedup vs baseline: 4.2947x; 1.1391x over previous
"""Trainium2 Bass kernel for nn_BSplineActivationLayer.

Math:  y[b,o] = softplus( (1/OUT) * sum_i G[o,i] * f(x[b,i]; b1..b5[o,i]) )
where G = softplus(raw_gamma), b_s = piecewise-cubic spline of
w_norm = (clip(w,5.5,35.5)-20)/9, and
  f(x; b) = b1*log1p(b2*log1p((exp(b3*x)-1)**b4)) + b5*x.

Device algorithm (per core, OUT sharded 8 ways), tuned to the 2e-2 rel-err
budget (measured end-to-end error ~2e-3):
  * spline b_s is approximated piecewise-CONSTANT per piece (value of the
    cubic at the piece midpoint t=0.125); the per-element piece gather is a
    12-step masked multiply-accumulate per spline with the table values
    baked into the instruction stream as immediates (compile happens after
    inputs are seen; cache keyed on the table bytes).  clip() bounds prove
    pieces 0 and 14 unreachable, and the breaks are uniform so the masks
    compare raw w against MU+SIG*brk_j directly -- no normalization ops.
  * f is analytic in u = log(x); interpolate at NN=4 Chebyshev nodes in u:
      y[b,o] = softplus( (1/OUT) * [ sum_m  L_m(v[b,i]) @ N_m[o,i]
                                     + x @ (G*b5)[o,i] ] )
    with N_m = G*b1*cm_m*log1p(b2*log1p((exp(b3*x_m)-1)**b4)) node values
    and L_m the (unscaled) Lagrange basis products of v = norm(log x).
  * work is balanced across DVE (3 gather planes, chain muls, Lagrange
    finals, EN), Pool/GPSIMD (step masks, 2 gather planes, DD, products,
    gammas -- walrus accepts TensorTensor and immediate TensorScalar on
    Pool), and ACT (one manually placed set-6 table load serves every
    exp/ln/copy).  Matmul operands round to bf16 except the x-term, which
    stays f32 (PE has slack).  Junk matmuls keep PE ramped before the tail.
All value-dependent math on the big tensors runs on device; the host only
shards / transposes inputs, prepares the tiny (5x15) spline table constants,
and concatenates outputs.
"""

import numpy as np

B, IN, OUT = 256, 512, 512
NCORES = 8
OSH = OUT // NCORES            # 64 out-rows per core
NN = 4                         # interpolation nodes
NPIECE = 15
MU, SIG = 20.0, 9.0
U_LO, U_HI = float(np.log(0.01)), float(np.log(1.011))
TM = 0.125                     # piece-midpoint for the constant approx
JLO, JHI = 2, 13               # reachable step boundaries
NWARM = 0                      # junk matmuls to keep PE ramped (tuned)

_CACHE = {}


def _nodes():
    k = np.arange(NN)
    vn = np.cos((2 * k + 1) * np.pi / (2 * NN))          # in (-1, 1)
    xn = np.exp(0.5 * (U_HI + U_LO) + 0.5 * (U_HI - U_LO) * vn)
    cm = np.array([1.0 / np.prod(vn[m] - np.delete(vn, m)) for m in range(NN)])
    return vn, xn, cm


def _tables(breaks, coefs):
    """Host prep of the small spline tables -> immediates.

    Returns thr[j] (mask thresholds in raw-w domain, j=JLO..JHI),
    base[s], delta[s][j] for the piecewise-constant masked accumulate."""
    brk = breaks[0].astype(np.float64)
    cf = coefs.astype(np.float64)
    a3, a2, a1, a0 = cf[..., 0], cf[..., 1], cf[..., 2], cf[..., 3]
    vmid = ((a3 * TM + a2) * TM + a1) * TM + a0          # [NS, K]
    thr = MU + SIG * brk                                  # [16]
    base = vmid[:, 1]
    delta = vmid[:, 1:] - vmid[:, :-1]                    # delta[s, j-1] = v_j - v_{j-1}
    return thr, base, vmid, delta


def _emit(ctx, tc, yT, xT, wT, rgT, thr, base, delta):
    import concourse.bass as bass
    from concourse import mybir

    nc = tc.nc
    f32 = mybir.dt.float32
    bf16 = mybir.dt.bfloat16
    Alu = mybir.AluOpType
    Act = mybir.ActivationFunctionType
    vn, xn, cm = _nodes()

    P = 128
    IC = IN // P                      # 4 i-chunks
    FO = IC * OSH                     # 256
    FB = IC * B                       # 1024

    pool = ctx.enter_context(tc.tile_pool(name="main", bufs=1))
    pps = ctx.enter_context(tc.tile_pool(name="ps", bufs=1, space="PSUM"))

    def bcast_mid(ap2d, n):
        a = ap2d
        return bass.AP(tensor=a.tensor, offset=a.offset,
                       ap=[a.ap[0], [0, n], a.ap[1]])

    V = nc.vector
    Pl = nc.gpsimd
    S_ = nc.scalar

    ascale = 2.0 / (U_HI - U_LO)
    boff = (U_HI + U_LO) / (U_HI - U_LO)

    # ---- constants ---------------------------------------------------
    CP1 = pool.tile([P, 1], f32)
    V.memset(CP1, 1.0)
    CN1 = pool.tile([P, 1], f32)
    V.memset(CN1, -1.0)

    # ---- manual act-table load: set 6 covers exp/ln/copy -------------
    atl = mybir.InstLoadActFuncSet(
        name=nc.get_next_instruction_name(), act_func_set_id=6, ins=[], outs=[])
    S_.add_instruction(atl)

    # ---- DMAs --------------------------------------------------------
    W = pool.tile([P, FO], f32)
    nc.sync.dma_start(out=W.rearrange("p (c o) -> p c o", c=IC), in_=bass.AP(
        tensor=wT.tensor, offset=wT.offset,
        ap=[[OSH, P], [P * OSH, IC], [1, OSH]]))
    X = pool.tile([P, IC, B], f32)
    nc.sync.dma_start(out=X, in_=bass.AP(
        tensor=xT.tensor, offset=xT.offset,
        ap=[[B, P], [P * B, IC], [1, B]]))
    RG = pool.tile([P, FO], f32)
    nc.sync.dma_start(out=RG.rearrange("p (c o) -> p c o", c=IC), in_=bass.AP(
        tensor=rgT.tensor, offset=rgT.offset,
        ap=[[OSH, P], [P * OSH, IC], [1, OSH]]))

    # ---- step masks (Pool) ------------------------------------------
    NSTEP = JHI - JLO + 1             # 12
    ST = pool.tile([P, NSTEP, FO], f32)
    for j in range(JLO, JHI + 1):
        Pl.tensor_scalar(ST[:, j - JLO, :], W, float(thr[j]), 1.0,
                         Alu.is_gt, Alu.mult)

    # ---- gamma + log(x) (ACT) ---------------------------------------
    G = pool.tile([P, FO], f32)
    S_.activation(G, RG, Act.Exp)
    S_.activation(G, G, Act.Ln, bias=CP1)         # softplus(rg)
    U = pool.tile([P, IC, B], f32)
    XF = X.rearrange("p c b -> p (c b)")
    UF = U.rearrange("p c b -> p (c b)")
    S_.activation(UF, XF, Act.Ln)

    # ---- DD_m = v - vn_m on ACT (bf16), bf16 x copy for the x-term ---
    DD = pool.tile([P, NN, FB], bf16)
    for m in range(NN):
        S_.activation(DD[:, m, :], UF, Act.Copy, scale=ascale,
                      bias=-(boff + float(vn[m])))
    XB = pool.tile([P, IC, B], bf16)
    S_.activation(XB.rearrange("p c b -> p (c b)"), XF, Act.Copy)

    # ---- all 5 gather planes on DVE (TSP/STT has no perf modes; DVE is
    # still the cheapest engine for the masked accumulate).  Chain planes
    # first (b3,b4,b2), Lagrange finals, then b1 (gamma), b5 (x-term). --
    A = [pool.tile([P, FO], f32, name=f"A{s}") for s in range(5)]
    E = pool.tile([P, NN, FO], f32)
    EF = E.rearrange("p n f -> p (n f)")
    EB = pool.tile([P, NN, FO], bf16)
    P01 = pool.tile([P, FB], bf16)
    P23 = pool.tile([P, FB], bf16)
    LB = pool.tile([P, NN, FB], bf16)
    GB1 = pool.tile([P, FO], f32)
    GCM = pool.tile([P, NN, FO], bf16)
    GB5 = pool.tile([P, FO], bf16)

    def plane(s):
        V.tensor_scalar(A[s], ST[:, 0, :], float(delta[s, JLO - 1]),
                        float(base[s]), Alu.mult, Alu.add)
        for j in range(JLO + 1, JHI + 1):
            V.scalar_tensor_tensor(A[s], ST[:, j - JLO, :],
                                   float(delta[s, j - 1]), A[s],
                                   Alu.mult, Alu.add)

    plane(2)
    # b3 ready: node exponentials + lam on ACT
    for m in range(NN):
        S_.activation(E[:, m, :], A[2], Act.Exp, scale=float(xn[m]))
    S_.activation(EF, EF, Act.Ln, bias=CN1)       # lam = ln(e^{b3 xm}-1)
    # Lagrange pairs on Pool as soon as DD lands
    Pl.tensor_tensor(P01, DD[:, 0, :], DD[:, 1, :], Alu.mult)
    Pl.tensor_tensor(P23, DD[:, 2, :], DD[:, 3, :], Alu.mult)

    plane(3)
    Pl.tensor_tensor(E, E, bcast_mid(A[3], NN), Alu.mult)   # T = lam*b4
    S_.activation(EF, EF, Act.Exp)                # (e^{b3 xm}-1)^{b4}
    S_.activation(EF, EF, Act.Ln, bias=CP1)       # L1 = log1p(...)

    plane(1)
    Pl.tensor_tensor(E, E, bcast_mid(A[1], NN), Alu.mult)   # b2*L1
    S_.activation(EB.rearrange("p n f -> p (n f)"), EF,
                  Act.Ln, bias=CP1)               # L2 = log1p(b2 L1)

    # Lagrange finals on DVE (bf16 2x) between chain planes and tail planes
    V.tensor_tensor(LB[:, 0, :], DD[:, 1, :], P23, Alu.mult)
    V.tensor_tensor(LB[:, 1, :], DD[:, 0, :], P23, Alu.mult)
    V.tensor_tensor(LB[:, 2, :], P01, DD[:, 3, :], Alu.mult)
    V.tensor_tensor(LB[:, 3, :], P01, DD[:, 2, :], Alu.mult)

    plane(0)
    Pl.tensor_tensor(GB1, G, A[0], Alu.mult)
    for m in range(NN):
        Pl.tensor_scalar(GCM[:, m, :], GB1, float(cm[m]), 1.0,
                         Alu.mult, Alu.mult)

    plane(4)
    Pl.tensor_tensor(GB5, G, A[4], Alu.mult)

    # ---- matmuls; EN split per node so PE starts as soon as possible -
    ps = pps.tile([OSH, B], f32)
    if NWARM:
        psj = pps.tile([OSH, B], f32)
        ZJ = pool.tile([P, OSH], bf16)
        ZM = pool.tile([P, B], bf16)
        V.memset(ZJ, 0.0)
        V.memset(ZM, 0.0)
        for k in range(NWARM):
            nc.tensor.matmul(psj, ZJ, ZM, start=(k == 0), stop=(k == NWARM - 1))
    nmm = IC * (NN + 1)
    k = 0
    EN = pool.tile([P, NN, FO], bf16)
    ENv = EN.rearrange("p n (c o) -> p n c o", c=IC)
    LBv = LB.rearrange("p n (c b) -> p n c b", c=IC)
    for m in range(NN):
        V.tensor_tensor(EN[:, m, :], EB[:, m, :], GCM[:, m, :], Alu.mult)
        for ic in range(IC):
            nc.tensor.matmul(ps, ENv[:, m, ic, :], LBv[:, m, ic, :],
                             start=(k == 0), stop=(k == nmm - 1))
            k += 1
    GB5v = GB5.rearrange("p (c o) -> p c o", c=IC)
    for ic in range(IC):
        nc.tensor.matmul(ps, GB5v[:, ic, :], XB[:, ic, :],
                         start=(k == 0), stop=(k == nmm - 1))
        k += 1

    # ---- softplus + store -------------------------------------------
    Y = pool.tile([OSH, B], f32)
    S_.activation(Y, ps, Act.Exp, scale=1.0 / OUT)
    S_.activation(Y, Y, Act.Ln, bias=CP1[0:OSH, :])
    nc.sync.dma_start(out=yT, in_=Y)


def _build(breaks, coefs):
    key = (breaks.tobytes(), coefs.tobytes())
    if key in _CACHE:
        return _CACHE[key]
    from contextlib import ExitStack
    import concourse.bacc as bacc
    import concourse.tile as tile
    from concourse import mybir

    thr, base, vmid, delta = _tables(breaks, coefs)

    f32 = mybir.dt.float32
    nc = bacc.Bacc("TRN2", target_bir_lowering=False, debug=False,
                   num_devices=NCORES)
    xT = nc.dram_tensor("xT", [IN, B], f32, kind="ExternalInput").ap()
    wT = nc.dram_tensor("wT", [IN, OSH], f32, kind="ExternalInput").ap()
    rgT = nc.dram_tensor("rgT", [IN, OSH], f32, kind="ExternalInput").ap()
    yT = nc.dram_tensor("yT", [OSH, B], f32, kind="ExternalOutput").ap()

    with tile.TileContext(nc) as tc, ExitStack() as ctx:
        _emit(ctx, tc, yT, xT, wT, rgT, thr, base, delta)
    nc.compile()
    _CACHE[key] = nc
    return nc


def _prep_inputs(x, raw_gamma, w, breaks, coefs):
    xT = np.ascontiguousarray(x.T, dtype=np.float32)
    maps = []
    for c in range(NCORES):
        o0, o1 = c * OSH, (c + 1) * OSH
        maps.append({
            "xT": xT,
            "wT": np.ascontiguousarray(w[o0:o1].T, dtype=np.float32),
            "rgT": np.ascontiguousarray(raw_gamma[o0:o1].T, dtype=np.float32),
        })
    return maps


def kernel(x, raw_gamma, w, breaks, coefs):
    from concourse.bass_utils import run_bass_kernel_spmd
    nc = _build(np.asarray(breaks), np.asarray(coefs))
    maps = _prep_inputs(x, raw_gamma, w, breaks, coefs)
    res = run_bass_kernel_spmd(nc, maps, list(range(NCORES)))
    y = np.concatenate([res.results[c]["yT"].T for c in range(NCORES)], axis=1)
    return np.ascontiguousarray(y, dtype=np.float32)
